# revision 9
# baseline (speedup 1.0000x reference)
"""Multi-head attention (B=2, S=2048, D=1024, H=16) on 8 Trainium2 NeuronCores.

Sharding: data-parallel over batch (2 groups of 4 cores) x tensor-parallel over
heads (4 heads / core). The wall clock is dominated by the host<->device tunnel,
so the design minimizes transferred bytes and per-call overhead:

  - Inputs ship as fp16 token shards, token-major (no host transpose; the
    device PE-transposes after an on-device AllGather reconstructs the full
    sequence within each 4-core batch group).
  - Weights ship fp16, split in half between paired cores (c, c+4) and
    reassembled with a 2-core AllGather — every weight byte crosses the
    tunnel exactly once.
  - All inputs are packed into 4 arrays (xqkv / w / b / idm) to amortize
    per-array dispatch+transfer overhead; host staging buffers are
    preallocated once and reused (no per-call page faults).
  - Each core's partial output projection is summed on device with a
    ReduceScatter; each core returns its 512 tokens of the final output in
    fp16. The donated output buffer from the previous call is recycled so
    no zero-buffer is uploaded.
  - The jitted PJRT executable is built once and cached.

Device kernel notes (per core):
  - Raw token-major fp16 chunks are transposed feature-major via PE identity
    matmuls (psum f32 -> fp16 copy), then projections consume fp16 operands;
    attention internals stay f32r (scores computed transposed, softmax
    without max-subtraction, denominator via a ones-column in the PV
    stationary operand, 1/l broadcast with a K=1 ones matmul).
  - The output projection is computed token-major so ReduceScatter chunks
    are token-contiguous and host reassembly is a plain cast.
"""

import numpy as np

D_MODEL = 1024
S = 2048
N_CORES = 8
HPC = 4           # heads per core
COF = HPC * 64    # 256 out-features per core
TPS = S // 4      # 512 tokens per shard
WBLK = D_MODEL * COF  # 262144 elems per weight slice
WHALF = 2 * WBLK      # per-core weight half

_CACHED = None


def _build():
    from concourse import bacc
    import concourse.bass as bass
    import concourse.tile as tile
    from concourse import mybir

    F16 = mybir.dt.float16
    F32R = mybir.dt.float32r
    F32 = mybir.dt.float32
    EXP = mybir.ActivationFunctionType.Exp

    nc = bacc.Bacc("TRN2", target_bir_lowering=False, debug=False,
                   num_devices=N_CORES)

    xqkv = nc.dram_tensor("xqkv", [3, TPS, D_MODEL], F16, kind="ExternalInput")
    w_in = nc.dram_tensor("w_in", [WHALF], F16, kind="ExternalInput")
    b_in = nc.dram_tensor("b_in", [832], F32R, kind="ExternalInput")
    idm = nc.dram_tensor("idm", [128, 128], F16, kind="ExternalInput")
    outp = nc.dram_tensor("outp", [TPS, D_MODEL], F16, kind="ExternalOutput")

    GROUPS4 = [[0, 1, 2, 3], [4, 5, 6, 7]]
    PAIRS = [[0, 4], [1, 5], [2, 6], [3, 7]]

    with nc.allow_low_precision(reason="fp16 transfers / f32r matmuls intended"), \
            tile.TileContext(nc) as tc:
        with (
            tc.tile_pool(name="dram", bufs=1, space="DRAM") as dram,
            tc.tile_pool(name="wconst", bufs=1) as wconst,
            tc.tile_pool(name="big", bufs=1) as big,
            tc.tile_pool(name="raw", bufs=2) as raw_pool,
            tc.tile_pool(name="qin", bufs=2) as qin_pool,
            tc.tile_pool(name="expp", bufs=4) as expp,
            tc.tile_pool(name="stage", bufs=3) as stage_pool,
            tc.tile_pool(name="bcp", bufs=2) as bcp,
            tc.tile_pool(name="small", bufs=4) as small,
            tc.tile_pool(name="psA", bufs=4, space="PSUM") as psA,
            tc.tile_pool(name="psS", bufs=2, space="PSUM") as psS,
        ):
            # ---- collective bounce buffers (DRAM) ----
            xin_b = dram.tile([3, TPS, D_MODEL], F16)    # my shard of q,k,v
            gX = dram.tile([4, 3, TPS, D_MODEL], F16)    # gathered full seq
            wb = dram.tile([WHALF], F16)                 # my weight half
            gW = dram.tile([2, WHALF], F16)              # full weight block
            ob_in = dram.tile([S, D_MODEL], F16)         # my partial out
            ob_out = dram.tile([TPS, D_MODEL], F16)      # reduced shard

            nc.sync.dma_start(xin_b[:], xqkv[:])
            nc.sync.dma_start(wb[:], w_in[:])
            nc.gpsimd.collective_compute(
                "AllGather", mybir.AluOpType.bypass,
                replica_groups=GROUPS4,
                ins=[xin_b[:].opt()], outs=[gX[:].opt()],
            )
            nc.gpsimd.collective_compute(
                "AllGather", mybir.AluOpType.bypass,
                replica_groups=PAIRS,
                ins=[wb[:].opt()], outs=[gW[:].opt()],
            )

            # ---- weights / biases to SBUF ----
            wq_sb = wconst.tile([128, 8, COF], F16)
            wk_sb = wconst.tile([128, 8, COF], F16)
            wv_sb = wconst.tile([128, 8, COF], F16)
            wo_sb = wconst.tile([128, 2, D_MODEL], F16)
            nc.sync.dma_start(
                wq_sb[:], gW[0, 0:WBLK].rearrange("(a p f) -> p a f", p=128, f=COF))
            nc.sync.dma_start(
                wk_sb[:], gW[0, WBLK:WHALF].rearrange("(a p f) -> p a f", p=128, f=COF))
            nc.sync.dma_start(
                wv_sb[:], gW[1, 0:WBLK].rearrange("(a p f) -> p a f", p=128, f=COF))
            nc.sync.dma_start(
                wo_sb[:], gW[1, WBLK:WHALF].rearrange("(c p f) -> p c f", p=128, f=D_MODEL))

            bq_sb = wconst.tile([128, 2], F32)
            bk_sb = wconst.tile([128, 2], F32)
            nc.sync.dma_start(
                bq_sb[:], b_in[0:256].rearrange("(p m) -> p m", m=2).bitcast(F32))
            nc.sync.dma_start(
                bk_sb[:], b_in[256:512].rearrange("(p m) -> p m", m=2).bitcast(F32))
            b_ap = b_in[:]
            bv_bc = wconst.tile([128, HPC, 64], F32)
            nc.gpsimd.dma_start(
                bv_bc[:],
                bass.AP(tensor=b_ap.tensor, offset=b_ap.offset + 512,
                        ap=[[0, 128], [64, HPC], [1, 64]]).bitcast(F32),
            )
            ones_sb = wconst.tile([1, 64], F32R)
            nc.sync.dma_start(ones_sb[:],
                              b_in[768:832].rearrange("(o c) -> o c", o=1))
            id_sb = wconst.tile([128, 128], F16)
            nc.sync.dma_start(id_sb[:], idm[:])

            # ---- persistent activations ----
            QT_sb = big.tile([128, 2, S], F32R)   # [p, m, t]: Q^T[m*128+p, t]
            KT_sb = big.tile([128, 2, S], F32R)
            V_sb = big.tile([128, 16, HPC, 65], F32R)  # [tok%128, tok//128, h, c]
            OT_sb = big.tile([128, 2, S], F16)    # normalized attention out^T

            # V ones-column (l accumulator rides along the PV matmul)
            for tt in range(16):
                nc.gpsimd.dma_start(
                    V_sb[:, tt, :, 64:65],
                    bass.AP(tensor=b_ap.tensor, offset=b_ap.offset + 768,
                            ap=[[0, 128], [0, HPC], [1, 1]]),
                )

            # ---- per-chunk: PE-transpose raw tokens, then project ----
            def proj_chunk(qc):
                # raw [tok%128, tokblk, featblk, feat] per tensor
                xin = qin_pool.tile([128, 3, 8, TPS], F16, tag="xin",
                                    name=f"xin_{qc}")
                for t in range(3):
                    rw = raw_pool.tile([128, 4, 8, 128], F16, tag="raw",
                                       name=f"raw_{qc}_{t}")
                    nc.sync.dma_start(
                        rw[:],
                        gX[qc, t].rearrange("(tb p) (fb f) -> p tb fb f",
                                            p=128, f=128),
                    )
                    for fb in range(8):
                        ps = psA.tile([128, 512], F32, tag="ps",
                                      name=f"tp_{qc}_{t}_{fb}")
                        for tb in range(4):
                            nc.tensor.matmul(
                                ps[:, tb * 128:(tb + 1) * 128],
                                rw[:, tb, fb, :], id_sb[:],
                                start=True, stop=True,
                            )
                        nc.vector.tensor_copy(xin[:, t, fb, :], ps[:])
                # Q/K projections (feature-major psum)
                for (ti, b_sb, dst) in ((0, bq_sb, QT_sb), (1, bk_sb, KT_sb)):
                    w_sb = wq_sb if ti == 0 else wk_sb
                    for m in range(2):
                        pq = psS.tile([128, 1024], F32, tag="sc",
                                      name=f"qkps_{qc}_{ti}_{m}")
                        for kt in range(8):
                            nc.tensor.matmul(
                                pq[:, 0:TPS],
                                w_sb[:, kt, m * 128:(m + 1) * 128],
                                xin[:, ti, kt, :],
                                start=(kt == 0), stop=(kt == 7),
                            )
                        nc.vector.tensor_scalar_add(
                            dst[:, m, qc * TPS:(qc + 1) * TPS], pq[:, 0:TPS],
                            b_sb[:, m:m + 1],
                        )
                # V projection (token-major psum)
                for tsub in range(4):
                    tt = qc * 4 + tsub
                    pv = psS.tile([128, 1024], F32, tag="sc",
                                  name=f"vps_{qc}_{tsub}")
                    for kt in range(8):
                        nc.tensor.matmul(
                            pv[:, 0:COF],
                            xin[:, 2, kt, tsub * 128:(tsub + 1) * 128],
                            wv_sb[:, kt, :],
                            start=(kt == 0), stop=(kt == 7),
                        )
                    nc.vector.tensor_add(
                        V_sb[:, tt, :, 0:64],
                        pv[:, 0:COF].rearrange("p (h c) -> p h c", h=HPC),
                        bv_bc[:],
                    )

            # ---- attention (baseline structure, f32r internals) ----
            def att_pass_alloc(hp, qh):
                return [[psA.tile([128, 512], F32, tag="ps",
                                  name=f"po_{hp}_{qh}_{h2}_{qcl}")
                         for qcl in range(2)] for h2 in range(2)]

            def att_ktgroup(hp, qh, po, kts):
                for kt in kts:
                    for h2 in range(2):
                        p0 = h2 * 64
                        sc = psS.tile([128, 1024], F32, tag="sc",
                                      name=f"sc_{hp}_{qh}_{kt}_{h2}")
                        for qcl in range(2):
                            qg = qh * 2 + qcl
                            nc.tensor.matmul(
                                sc[:, qcl * 512:(qcl + 1) * 512],
                                KT_sb[p0:p0 + 64, hp, kt * 128:(kt + 1) * 128],
                                QT_sb[p0:p0 + 64, hp, qg * 512:(qg + 1) * 512],
                                start=True, stop=True,
                                tile_position=(p0, 0),
                            )
                        ex = expp.tile([128, 1024], F32R, tag="ex",
                                       name=f"ex_{hp}_{qh}_{kt}_{h2}")
                        nc.scalar.activation(out=ex[:], in_=sc[:], func=EXP,
                                             scale=0.125)
                        for qcl in range(2):
                            nc.tensor.matmul(
                                po[h2][qcl][0:65, :],
                                V_sb[:, kt, hp * 2 + h2, :],
                                ex[:, qcl * 512:(qcl + 1) * 512],
                                start=(kt == 0), stop=(kt == 15),
                            )

            def att_norm(hp, qh, po):
                # OT = po[0:64] / l  (l rides in po row 64)
                for h2 in range(2):
                    for qcl in range(2):
                        qg = qh * 2 + qcl
                        p = po[h2][qcl]
                        linv = small.tile([1, 512], F32R, tag="linv",
                                          name=f"linv_{hp}_{qh}_{h2}_{qcl}")
                        nc.vector.reciprocal(linv[:], p[64:65, :])
                        bc_ps = psS.tile([64, 512], F32, tag="sc",
                                         name=f"bc_{hp}_{qh}_{h2}_{qcl}")
                        nc.tensor.matmul(
                            bc_ps[:], ones_sb[:], linv[:],
                            start=True, stop=True,
                        )
                        bc_sb = bcp.tile([64, 512], F32, tag="bc",
                                         name=f"bcs_{hp}_{qh}_{h2}_{qcl}")
                        nc.vector.tensor_copy(bc_sb[:], bc_ps[:])
                        nc.vector.tensor_mul(
                            OT_sb[h2 * 64:(h2 + 1) * 64, hp,
                                  qg * 512:(qg + 1) * 512],
                            p[0:64, :], bc_sb[:],
                        )

            def outproj_half(qh):
                # token-major partial: out[t, of] = OT[:, t].T @ wo  (256 feats)
                for tb in range(8):
                    tok0 = qh * 1024 + tb * 128
                    pg = [psA.tile([128, 512], F32, tag="ps",
                                   name=f"pg_{qh}_{tb}_{i}") for i in range(2)]
                    for ct in range(2):
                        for i in range(2):
                            nc.tensor.matmul(
                                pg[i][:],
                                OT_sb[:, ct, tok0:tok0 + 128],
                                wo_sb[:, ct, i * 512:(i + 1) * 512],
                                start=(ct == 0), stop=(ct == 1),
                            )
                    st = stage_pool.tile([128, 1024], F16, tag="st",
                                         name=f"st_{qh}_{tb}")
                    for i in range(2):
                        nc.vector.tensor_copy(st[:, i * 512:(i + 1) * 512],
                                              pg[i][:])
                    nc.sync.dma_start(ob_in[tok0:tok0 + 128, :], st[:])

            # ---- schedule (sequential; tunnel dominates, not device) ----
            for qc in range(4):
                proj_chunk(qc)
            for qh in range(2):
                for hp in range(2):
                    po = att_pass_alloc(hp, qh)
                    att_ktgroup(hp, qh, po, range(16))
                    att_norm(hp, qh, po)
                outproj_half(qh)

            nc.gpsimd.collective_compute(
                "ReduceScatter", mybir.AluOpType.add,
                replica_groups=GROUPS4,
                ins=[ob_in[:].opt()], outs=[ob_out[:].opt()],
            )
            nc.sync.dma_start(outp[:], ob_out[:])

    nc.compile()
    return nc


def _get_runner():
    """Build the bass program, cached jitted PJRT executable, and reusable
    host staging buffers once."""
    global _CACHED
    if _CACHED is not None:
        return _CACHED

    import jax
    from jax.sharding import Mesh, PartitionSpec
    from jax.experimental.shard_map import shard_map
    from concourse import mybir
    from concourse.bass2jax import (_bass_exec_p, install_neuronx_cc_hook,
                                    partition_id_tensor)

    nc = _build()
    install_neuronx_cc_hook()

    partition_name = (nc.partition_id_tensor.name
                      if nc.partition_id_tensor else None)
    in_names, out_names, out_avals, zero_shapes = [], [], [], []
    for alloc in nc.m.functions[0].allocations:
        if not isinstance(alloc, mybir.MemoryLocationSet):
            continue
        name = alloc.memorylocations[0].name
        if alloc.kind == "ExternalInput":
            if name != partition_name:
                in_names.append(name)
        elif alloc.kind == "ExternalOutput":
            shape = tuple(alloc.tensor_shape)
            dtype = mybir.dt.np(alloc.dtype)
            out_names.append(name)
            out_avals.append(jax.core.ShapedArray(shape, dtype))
            zero_shapes.append(((N_CORES * shape[0],) + shape[1:], dtype))
    n_params = len(in_names)
    n_outs = len(out_names)
    in_names_all = in_names + out_names + (
        [partition_name] if partition_name else [])

    def _body(*args):
        operands = list(args)
        if partition_name is not None:
            operands.append(partition_id_tensor())
        outs = _bass_exec_p.bind(
            *operands, out_avals=tuple(out_avals),
            in_names=tuple(in_names_all), out_names=tuple(out_names),
            lowering_input_output_aliases=(), sim_require_finite=True,
            sim_require_nnan=True, nc=nc)
        return tuple(outs)

    devices = jax.devices()[:N_CORES]
    mesh = Mesh(np.asarray(devices), ("core",))
    in_specs = (PartitionSpec("core"),) * (n_params + n_outs)
    out_specs = (PartitionSpec("core"),) * n_outs
    donate = tuple(range(n_params, n_params + n_outs))
    sharded = jax.jit(shard_map(_body, mesh=mesh, in_specs=in_specs,
                                out_specs=out_specs, check_rep=False),
                      donate_argnums=donate, keep_unused=True)

    # async H2D movers: dispatching a passthrough jit starts the transfer
    # immediately and returns; host prep of later arrays overlaps it
    shard1 = jax.sharding.NamedSharding(mesh, PartitionSpec("core"))
    pass1 = jax.jit(lambda x: x, out_shardings=shard1)
    pass2 = jax.jit(lambda x, y: (x, y), out_shardings=(shard1, shard1))

    # preallocated host staging buffers (reused across calls)
    f16 = np.float16
    stage = {
        "xq8": np.zeros((2, 4, 3, TPS, D_MODEL), f16),
        "w8": np.zeros((2, 4, WHALF), f16),
        "b8": np.zeros((2, 4, 832), np.float32),
    }
    id8 = np.zeros((N_CORES * 128, 128), f16)
    id8.reshape(N_CORES, 128, 128)[:] = np.eye(128, dtype=f16)
    id_dev = pass1(id8)  # identity is constant: resides on device forever

    _CACHED = dict(sharded=sharded, in_names=in_names,
                   zero_shapes=zero_shapes, out_names=out_names,
                   stage=stage, prev_out=None, pass1=pass1, pass2=pass2,
                   id_dev=id_dev)
    return _CACHED


def kernel(q, k, v, w_q, b_q, w_k, b_k, w_v, b_v, w_o, b_o):
    q, k, v = (np.asarray(x, np.float32) for x in (q, k, v))
    w_q, b_q, w_k, b_k, w_v, b_v, w_o, b_o = (
        np.asarray(x, np.float32)
        for x in (w_q, b_q, w_k, b_k, w_v, b_v, w_o, b_o)
    )

    r = _get_runner()
    st = r["stage"]

    # weights first: their H2D overlaps the qkv host prep below.
    # full block per head group = [wq_sl|wk_sl|wv_sl|wo_sl] flat;
    # core c gets half c//4 of its head group's block
    w8 = st["w8"]
    wf = w8.reshape(2, 4, 2, WBLK)  # [half, hg, (sub-half of pair), WBLK]
    # half 0 of the pair = wq|wk, half 1 = wv|wo
    wf[0, :, 0].reshape(4, D_MODEL, COF)[:] = (
        w_q.reshape(D_MODEL, 4, COF).transpose(1, 0, 2))
    wf[0, :, 1].reshape(4, D_MODEL, COF)[:] = (
        w_k.reshape(D_MODEL, 4, COF).transpose(1, 0, 2))
    wf[1, :, 0].reshape(4, D_MODEL, COF)[:] = (
        w_v.reshape(D_MODEL, 4, COF).transpose(1, 0, 2))
    wf[1, :, 1].reshape(4, COF, D_MODEL)[:] = w_o.reshape(4, COF, D_MODEL)

    # biases: [0:256] bq p-major, [256:512] bk, [512:768] bv, [768:832] ones
    b8 = st["b8"]
    b8[:, :, 0:256].reshape(2, 4, 128, 2)[:] = (
        b_q.reshape(4, 2, 128).transpose(0, 2, 1))
    b8[:, :, 256:512].reshape(2, 4, 128, 2)[:] = (
        b_k.reshape(4, 2, 128).transpose(0, 2, 1))
    b8[:, :, 512:768] = b_v.reshape(4, 256)
    b8[:, :, 768:832] = 1.0

    w_dev, b_dev = r["pass2"](w8.reshape(N_CORES * WHALF),
                              b8.reshape(N_CORES * 832))

    # xqkv: [core=(b,ts)][3, 512, 1024] token-major fp16 (prep overlaps the
    # weight transfer dispatched above)
    xq8 = st["xq8"]
    xq8[:, :, 0] = q.reshape(2, 4, TPS, D_MODEL)
    xq8[:, :, 1] = k.reshape(2, 4, TPS, D_MODEL)
    xq8[:, :, 2] = v.reshape(2, 4, TPS, D_MODEL)
    x_dev = r["pass1"](xq8.reshape(N_CORES * 3, TPS, D_MODEL))

    xs = {"xqkv": x_dev, "w_in": w_dev, "b_in": b_dev, "idm": r["id_dev"]}
    concat_in = [xs[nm] for nm in r["in_names"]]
    if r["prev_out"] is None:
        donated = [np.zeros(shape, dt) for shape, dt in r["zero_shapes"]]
    else:
        donated = [r["prev_out"]]
    out_arrs = r["sharded"](*concat_in, *donated)
    r["prev_out"] = out_arrs[0]

    # fetch shards in parallel; shard c = tokens (c%4)*512.. of batch c//4
    out = np.empty((2, S, D_MODEL), np.float32)
    out4 = out.reshape(N_CORES, TPS, D_MODEL)
    shards = sorted(out_arrs[0].addressable_shards,
                    key=lambda sh: sh.index[0].start or 0)
    for sh in shards:
        sh.data.copy_to_host_async()
    for i, sh in enumerate(shards):
        out4[i] = np.asarray(sh.data)
    out += b_o
    return out


# revision 11
# speedup vs baseline: 5.2667x; 5.2667x over previous
"""Multi-head attention (B=2, S=2048, D=1024, H=16) on 8 Trainium2 NeuronCores.

Sharding: data-parallel over batch (2 groups of 4 cores) x tensor-parallel over
heads (4 heads / core). The wall clock is dominated by the host<->device tunnel,
so the design minimizes transferred bytes and per-call overhead:

  - Inputs ship as fp16 token shards, token-major (no host transpose; the
    device PE-transposes after an on-device AllGather reconstructs the full
    sequence within each 4-core batch group).
  - Weights ship fp16, split in half between paired cores (c, c+4) and
    reassembled with a 2-core AllGather — every weight byte crosses the
    tunnel exactly once.
  - All inputs are packed into 4 arrays (xqkv / w / b / idm) to amortize
    per-array dispatch+transfer overhead; host staging buffers are
    preallocated once and reused (no per-call page faults).
  - Each core's partial output projection is summed on device with a
    ReduceScatter; each core returns its 512 tokens of the final output in
    fp16. The donated output buffer from the previous call is recycled so
    no zero-buffer is uploaded.
  - The jitted PJRT executable is built once and cached.

Device kernel notes (per core):
  - Raw token-major fp16 chunks are transposed feature-major via PE identity
    matmuls (psum f32 -> fp16 copy), then projections consume fp16 operands;
    attention internals stay f32r (scores computed transposed, softmax
    without max-subtraction, denominator via a ones-column in the PV
    stationary operand, 1/l broadcast with a K=1 ones matmul).
  - The output projection is computed token-major so ReduceScatter chunks
    are token-contiguous and host reassembly is a plain cast.
"""

import numpy as np

D_MODEL = 1024
S = 2048
N_CORES = 8
HPC = 4           # heads per core
COF = HPC * 64    # 256 out-features per core
TPS = S // 4      # 512 tokens per shard
WBLK = D_MODEL * COF  # 262144 elems per weight slice
WHALF = 2 * WBLK      # per-core weight half

_CACHED = None


def _build():
    from concourse import bacc
    import concourse.bass as bass
    import concourse.tile as tile
    from concourse import mybir

    F16 = mybir.dt.float16
    F32R = mybir.dt.float32r
    F32 = mybir.dt.float32
    EXP = mybir.ActivationFunctionType.Exp

    nc = bacc.Bacc("TRN2", target_bir_lowering=False, debug=False,
                   num_devices=N_CORES)

    xqkv = nc.dram_tensor("xqkv", [3, TPS, D_MODEL], F16, kind="ExternalInput")
    w_in = nc.dram_tensor("w_in", [WHALF], F16, kind="ExternalInput")
    b_in = nc.dram_tensor("b_in", [832], F32R, kind="ExternalInput")
    idm = nc.dram_tensor("idm", [128, 128], F16, kind="ExternalInput")
    outp = nc.dram_tensor("outp", [TPS, D_MODEL], F16, kind="ExternalOutput")

    GROUPS4 = [[0, 1, 2, 3], [4, 5, 6, 7]]
    PAIRS = [[0, 4], [1, 5], [2, 6], [3, 7]]

    with nc.allow_low_precision(reason="fp16 transfers / f32r matmuls intended"), \
            tile.TileContext(nc) as tc:
        with (
            tc.tile_pool(name="dram", bufs=1, space="DRAM") as dram,
            tc.tile_pool(name="wconst", bufs=1) as wconst,
            tc.tile_pool(name="big", bufs=1) as big,
            tc.tile_pool(name="raw", bufs=2) as raw_pool,
            tc.tile_pool(name="qin", bufs=2) as qin_pool,
            tc.tile_pool(name="expp", bufs=4) as expp,
            tc.tile_pool(name="stage", bufs=3) as stage_pool,
            tc.tile_pool(name="bcp", bufs=2) as bcp,
            tc.tile_pool(name="small", bufs=4) as small,
            tc.tile_pool(name="psA", bufs=4, space="PSUM") as psA,
            tc.tile_pool(name="psS", bufs=2, space="PSUM") as psS,
        ):
            # ---- collective bounce buffers (DRAM) ----
            xin_b = dram.tile([3, TPS, D_MODEL], F16)    # my shard of q,k,v
            gX = dram.tile([4, 3, TPS, D_MODEL], F16)    # gathered full seq
            wb = dram.tile([WHALF], F16)                 # my weight half
            gW = dram.tile([2, WHALF], F16)              # full weight block
            ob_in = dram.tile([S, D_MODEL], F16)         # my partial out
            ob_out = dram.tile([TPS, D_MODEL], F16)      # reduced shard

            nc.sync.dma_start(xin_b[:], xqkv[:])
            nc.sync.dma_start(wb[:], w_in[:])
            nc.gpsimd.collective_compute(
                "AllGather", mybir.AluOpType.bypass,
                replica_groups=GROUPS4,
                ins=[xin_b[:].opt()], outs=[gX[:].opt()],
            )
            nc.gpsimd.collective_compute(
                "AllGather", mybir.AluOpType.bypass,
                replica_groups=PAIRS,
                ins=[wb[:].opt()], outs=[gW[:].opt()],
            )

            # ---- weights / biases to SBUF ----
            wq_sb = wconst.tile([128, 8, COF], F16)
            wk_sb = wconst.tile([128, 8, COF], F16)
            wv_sb = wconst.tile([128, 8, COF], F16)
            wo_sb = wconst.tile([128, 2, D_MODEL], F16)
            nc.sync.dma_start(
                wq_sb[:], gW[0, 0:WBLK].rearrange("(a p f) -> p a f", p=128, f=COF))
            nc.sync.dma_start(
                wk_sb[:], gW[0, WBLK:WHALF].rearrange("(a p f) -> p a f", p=128, f=COF))
            nc.sync.dma_start(
                wv_sb[:], gW[1, 0:WBLK].rearrange("(a p f) -> p a f", p=128, f=COF))
            nc.sync.dma_start(
                wo_sb[:], gW[1, WBLK:WHALF].rearrange("(c p f) -> p c f", p=128, f=D_MODEL))

            bq_sb = wconst.tile([128, 2], F32)
            bk_sb = wconst.tile([128, 2], F32)
            nc.sync.dma_start(
                bq_sb[:], b_in[0:256].rearrange("(p m) -> p m", m=2).bitcast(F32))
            nc.sync.dma_start(
                bk_sb[:], b_in[256:512].rearrange("(p m) -> p m", m=2).bitcast(F32))
            b_ap = b_in[:]
            bv_bc = wconst.tile([128, HPC, 64], F32)
            nc.gpsimd.dma_start(
                bv_bc[:],
                bass.AP(tensor=b_ap.tensor, offset=b_ap.offset + 512,
                        ap=[[0, 128], [64, HPC], [1, 64]]).bitcast(F32),
            )
            ones_sb = wconst.tile([1, 64], F32R)
            nc.sync.dma_start(ones_sb[:],
                              b_in[768:832].rearrange("(o c) -> o c", o=1))
            id_sb = wconst.tile([128, 128], F16)
            nc.sync.dma_start(id_sb[:], idm[:])

            # ---- persistent activations ----
            QT_sb = big.tile([128, 2, S], F32R)   # [p, m, t]: Q^T[m*128+p, t]
            KT_sb = big.tile([128, 2, S], F32R)
            V_sb = big.tile([128, 16, HPC, 65], F32R)  # [tok%128, tok//128, h, c]
            OT_sb = big.tile([128, 2, S], F16)    # normalized attention out^T

            # V ones-column (l accumulator rides along the PV matmul)
            for tt in range(16):
                nc.gpsimd.dma_start(
                    V_sb[:, tt, :, 64:65],
                    bass.AP(tensor=b_ap.tensor, offset=b_ap.offset + 768,
                            ap=[[0, 128], [0, HPC], [1, 1]]),
                )

            # ---- per-chunk: PE-transpose raw tokens, then project ----
            def proj_chunk(qc):
                # raw [tok%128, tokblk, featblk, feat] per tensor
                xin = qin_pool.tile([128, 3, 8, TPS], F16, tag="xin",
                                    name=f"xin_{qc}")
                for t in range(3):
                    rw = raw_pool.tile([128, 4, 8, 128], F16, tag="raw",
                                       name=f"raw_{qc}_{t}")
                    nc.sync.dma_start(
                        rw[:],
                        gX[qc, t].rearrange("(tb p) (fb f) -> p tb fb f",
                                            p=128, f=128),
                    )
                    for fb in range(8):
                        ps = psA.tile([128, 512], F32, tag="ps",
                                      name=f"tp_{qc}_{t}_{fb}")
                        for tb in range(4):
                            nc.tensor.matmul(
                                ps[:, tb * 128:(tb + 1) * 128],
                                rw[:, tb, fb, :], id_sb[:],
                                start=True, stop=True,
                            )
                        nc.vector.tensor_copy(xin[:, t, fb, :], ps[:])
                # Q/K projections (feature-major psum)
                for (ti, b_sb, dst) in ((0, bq_sb, QT_sb), (1, bk_sb, KT_sb)):
                    w_sb = wq_sb if ti == 0 else wk_sb
                    for m in range(2):
                        pq = psS.tile([128, 1024], F32, tag="sc",
                                      name=f"qkps_{qc}_{ti}_{m}")
                        for kt in range(8):
                            nc.tensor.matmul(
                                pq[:, 0:TPS],
                                w_sb[:, kt, m * 128:(m + 1) * 128],
                                xin[:, ti, kt, :],
                                start=(kt == 0), stop=(kt == 7),
                            )
                        nc.vector.tensor_scalar_add(
                            dst[:, m, qc * TPS:(qc + 1) * TPS], pq[:, 0:TPS],
                            b_sb[:, m:m + 1],
                        )
                # V projection (token-major psum)
                for tsub in range(4):
                    tt = qc * 4 + tsub
                    pv = psS.tile([128, 1024], F32, tag="sc",
                                  name=f"vps_{qc}_{tsub}")
                    for kt in range(8):
                        nc.tensor.matmul(
                            pv[:, 0:COF],
                            xin[:, 2, kt, tsub * 128:(tsub + 1) * 128],
                            wv_sb[:, kt, :],
                            start=(kt == 0), stop=(kt == 7),
                        )
                    nc.vector.tensor_add(
                        V_sb[:, tt, :, 0:64],
                        pv[:, 0:COF].rearrange("p (h c) -> p h c", h=HPC),
                        bv_bc[:],
                    )

            # ---- attention (baseline structure, f32r internals) ----
            def att_pass_alloc(hp, qh):
                return [[psA.tile([128, 512], F32, tag="ps",
                                  name=f"po_{hp}_{qh}_{h2}_{qcl}")
                         for qcl in range(2)] for h2 in range(2)]

            def att_ktgroup(hp, qh, po, kts):
                for kt in kts:
                    for h2 in range(2):
                        p0 = h2 * 64
                        sc = psS.tile([128, 1024], F32, tag="sc",
                                      name=f"sc_{hp}_{qh}_{kt}_{h2}")
                        for qcl in range(2):
                            qg = qh * 2 + qcl
                            nc.tensor.matmul(
                                sc[:, qcl * 512:(qcl + 1) * 512],
                                KT_sb[p0:p0 + 64, hp, kt * 128:(kt + 1) * 128],
                                QT_sb[p0:p0 + 64, hp, qg * 512:(qg + 1) * 512],
                                start=True, stop=True,
                                tile_position=(p0, 0),
                            )
                        ex = expp.tile([128, 1024], F32R, tag="ex",
                                       name=f"ex_{hp}_{qh}_{kt}_{h2}")
                        nc.scalar.activation(out=ex[:], in_=sc[:], func=EXP,
                                             scale=0.125)
                        for qcl in range(2):
                            nc.tensor.matmul(
                                po[h2][qcl][0:65, :],
                                V_sb[:, kt, hp * 2 + h2, :],
                                ex[:, qcl * 512:(qcl + 1) * 512],
                                start=(kt == 0), stop=(kt == 15),
                            )

            def att_norm(hp, qh, po):
                # OT = po[0:64] / l  (l rides in po row 64)
                for h2 in range(2):
                    for qcl in range(2):
                        qg = qh * 2 + qcl
                        p = po[h2][qcl]
                        linv = small.tile([1, 512], F32R, tag="linv",
                                          name=f"linv_{hp}_{qh}_{h2}_{qcl}")
                        nc.vector.reciprocal(linv[:], p[64:65, :])
                        bc_ps = psS.tile([64, 512], F32, tag="sc",
                                         name=f"bc_{hp}_{qh}_{h2}_{qcl}")
                        nc.tensor.matmul(
                            bc_ps[:], ones_sb[:], linv[:],
                            start=True, stop=True,
                        )
                        bc_sb = bcp.tile([64, 512], F32, tag="bc",
                                         name=f"bcs_{hp}_{qh}_{h2}_{qcl}")
                        nc.vector.tensor_copy(bc_sb[:], bc_ps[:])
                        nc.vector.tensor_mul(
                            OT_sb[h2 * 64:(h2 + 1) * 64, hp,
                                  qg * 512:(qg + 1) * 512],
                            p[0:64, :], bc_sb[:],
                        )

            def outproj_half(qh):
                # token-major partial: out[t, of] = OT[:, t].T @ wo  (256 feats)
                for tb in range(8):
                    tok0 = qh * 1024 + tb * 128
                    pg = [psA.tile([128, 512], F32, tag="ps",
                                   name=f"pg_{qh}_{tb}_{i}") for i in range(2)]
                    for ct in range(2):
                        for i in range(2):
                            nc.tensor.matmul(
                                pg[i][:],
                                OT_sb[:, ct, tok0:tok0 + 128],
                                wo_sb[:, ct, i * 512:(i + 1) * 512],
                                start=(ct == 0), stop=(ct == 1),
                            )
                    st = stage_pool.tile([128, 1024], F16, tag="st",
                                         name=f"st_{qh}_{tb}")
                    for i in range(2):
                        nc.vector.tensor_copy(st[:, i * 512:(i + 1) * 512],
                                              pg[i][:])
                    nc.sync.dma_start(ob_in[tok0:tok0 + 128, :], st[:])

            # ---- schedule (sequential; tunnel dominates, not device) ----
            for qc in range(4):
                proj_chunk(qc)
            for qh in range(2):
                for hp in range(2):
                    po = att_pass_alloc(hp, qh)
                    att_ktgroup(hp, qh, po, range(16))
                    att_norm(hp, qh, po)
                outproj_half(qh)

            nc.gpsimd.collective_compute(
                "ReduceScatter", mybir.AluOpType.add,
                replica_groups=GROUPS4,
                ins=[ob_in[:].opt()], outs=[ob_out[:].opt()],
            )
            nc.sync.dma_start(outp[:], ob_out[:])

    nc.compile()
    return nc


def _get_runner():
    """Build the bass program, cached jitted PJRT executable, and reusable
    host staging buffers once."""
    global _CACHED
    if _CACHED is not None:
        return _CACHED

    import jax
    from jax.sharding import Mesh, PartitionSpec
    from jax.experimental.shard_map import shard_map
    from concourse import mybir
    from concourse.bass2jax import (_bass_exec_p, install_neuronx_cc_hook,
                                    partition_id_tensor)

    nc = _build()
    install_neuronx_cc_hook()

    partition_name = (nc.partition_id_tensor.name
                      if nc.partition_id_tensor else None)
    in_names, out_names, out_avals, zero_shapes = [], [], [], []
    for alloc in nc.m.functions[0].allocations:
        if not isinstance(alloc, mybir.MemoryLocationSet):
            continue
        name = alloc.memorylocations[0].name
        if alloc.kind == "ExternalInput":
            if name != partition_name:
                in_names.append(name)
        elif alloc.kind == "ExternalOutput":
            shape = tuple(alloc.tensor_shape)
            dtype = mybir.dt.np(alloc.dtype)
            out_names.append(name)
            out_avals.append(jax.core.ShapedArray(shape, dtype))
            zero_shapes.append(((N_CORES * shape[0],) + shape[1:], dtype))
    n_params = len(in_names)
    n_outs = len(out_names)
    in_names_all = in_names + out_names + (
        [partition_name] if partition_name else [])

    def _body(*args):
        operands = list(args)
        if partition_name is not None:
            operands.append(partition_id_tensor())
        outs = _bass_exec_p.bind(
            *operands, out_avals=tuple(out_avals),
            in_names=tuple(in_names_all), out_names=tuple(out_names),
            lowering_input_output_aliases=(), sim_require_finite=True,
            sim_require_nnan=True, nc=nc)
        return tuple(outs)

    devices = jax.devices()[:N_CORES]
    mesh = Mesh(np.asarray(devices), ("core",))
    in_specs = (PartitionSpec("core"),) * (n_params + n_outs)
    out_specs = (PartitionSpec("core"),) * n_outs
    donate = tuple(range(n_params, n_params + n_outs))
    sharded = jax.jit(shard_map(_body, mesh=mesh, in_specs=in_specs,
                                out_specs=out_specs, check_rep=False),
                      donate_argnums=donate, keep_unused=True)

    # preallocated host staging buffers (reused across calls)
    f16 = np.float16
    stage = {
        "xq8": np.zeros((2, 4, 3, TPS, D_MODEL), f16),
        "w8": np.zeros((2, 4, WHALF), f16),
        "b8": np.zeros((2, 4, 832), np.float32),
        "id8": np.zeros((N_CORES * 128, 128), f16),
    }
    stage["id8"].reshape(N_CORES, 128, 128)[:] = np.eye(128, dtype=f16)

    _CACHED = dict(sharded=sharded, in_names=in_names,
                   zero_shapes=zero_shapes, out_names=out_names,
                   stage=stage, prev_out=None)
    return _CACHED


def kernel(q, k, v, w_q, b_q, w_k, b_k, w_v, b_v, w_o, b_o):
    q, k, v = (np.asarray(x, np.float32) for x in (q, k, v))
    w_q, b_q, w_k, b_k, w_v, b_v, w_o, b_o = (
        np.asarray(x, np.float32)
        for x in (w_q, b_q, w_k, b_k, w_v, b_v, w_o, b_o)
    )

    r = _get_runner()
    st = r["stage"]

    # weights first: their H2D overlaps the qkv host prep below.
    # full block per head group = [wq_sl|wk_sl|wv_sl|wo_sl] flat;
    # core c gets half c//4 of its head group's block
    w8 = st["w8"]
    wf = w8.reshape(2, 4, 2, WBLK)  # [half, hg, (sub-half of pair), WBLK]
    # half 0 of the pair = wq|wk, half 1 = wv|wo
    wf[0, :, 0].reshape(4, D_MODEL, COF)[:] = (
        w_q.reshape(D_MODEL, 4, COF).transpose(1, 0, 2))
    wf[0, :, 1].reshape(4, D_MODEL, COF)[:] = (
        w_k.reshape(D_MODEL, 4, COF).transpose(1, 0, 2))
    wf[1, :, 0].reshape(4, D_MODEL, COF)[:] = (
        w_v.reshape(D_MODEL, 4, COF).transpose(1, 0, 2))
    wf[1, :, 1].reshape(4, COF, D_MODEL)[:] = w_o.reshape(4, COF, D_MODEL)

    # biases: [0:256] bq p-major, [256:512] bk, [512:768] bv, [768:832] ones
    b8 = st["b8"]
    b8[:, :, 0:256].reshape(2, 4, 128, 2)[:] = (
        b_q.reshape(4, 2, 128).transpose(0, 2, 1))
    b8[:, :, 256:512].reshape(2, 4, 128, 2)[:] = (
        b_k.reshape(4, 2, 128).transpose(0, 2, 1))
    b8[:, :, 512:768] = b_v.reshape(4, 256)
    b8[:, :, 768:832] = 1.0

    # xqkv: [core=(b,ts)][3, 512, 1024] token-major fp16
    xq8 = st["xq8"]
    xq8[:, :, 0] = q.reshape(2, 4, TPS, D_MODEL)
    xq8[:, :, 1] = k.reshape(2, 4, TPS, D_MODEL)
    xq8[:, :, 2] = v.reshape(2, 4, TPS, D_MODEL)

    xs = {
        "xqkv": xq8.reshape(N_CORES * 3, TPS, D_MODEL),
        "w_in": w8.reshape(N_CORES * WHALF),
        "b_in": b8.reshape(N_CORES * 832),
        "idm": st["id8"],
    }
    concat_in = [xs[nm] for nm in r["in_names"]]
    if r["prev_out"] is None:
        donated = [np.zeros(shape, dt) for shape, dt in r["zero_shapes"]]
    else:
        donated = [r["prev_out"]]
    out_arrs = r["sharded"](*concat_in, *donated)
    r["prev_out"] = out_arrs[0]

    # fetch shards in parallel; shard c = tokens (c%4)*512.. of batch c//4
    out = np.empty((2, S, D_MODEL), np.float32)
    out4 = out.reshape(N_CORES, TPS, D_MODEL)
    shards = sorted(out_arrs[0].addressable_shards,
                    key=lambda sh: sh.index[0].start or 0)
    for sh in shards:
        sh.data.copy_to_host_async()
    for i, sh in enumerate(shards):
        out4[i] = np.asarray(sh.data)
    out += b_o
    return out


# revision 22
# speedup vs baseline: 5.5224x; 1.0486x over previous
"""Multi-head attention (B=2, S=2048, D=1024, H=16) on 8 Trainium2 NeuronCores.

Sharding: data-parallel over batch (2 groups of 4 cores) x tensor-parallel over
heads (4 heads / core). The wall clock is dominated by the host<->device tunnel,
so the design minimizes transferred bytes and per-call overhead:

  - Inputs ship as fp16 token shards, token-major (no host transpose; the
    device PE-transposes after an on-device AllGather reconstructs the full
    sequence within each 4-core batch group).
  - Weights ship fp16, split in half between paired cores (c, c+4) and
    reassembled with a 2-core AllGather — every weight byte crosses the
    tunnel exactly once.
  - All inputs are packed into 4 arrays (xqkv / w / b / idm) to amortize
    per-array dispatch+transfer overhead; host staging buffers are
    preallocated once and reused (no per-call page faults).
  - Each core's partial output projection is summed on device with a
    ReduceScatter; each core returns its 512 tokens of the final output in
    fp16. The donated output buffer from the previous call is recycled so
    no zero-buffer is uploaded.
  - The jitted PJRT executable is built once and cached.

Device kernel notes (per core):
  - Raw token-major fp16 chunks are transposed feature-major via PE identity
    matmuls (psum f32 -> fp16 copy), then projections consume fp16 operands;
    attention internals stay f32r (scores computed transposed, softmax
    without max-subtraction, denominator via a ones-column in the PV
    stationary operand, 1/l broadcast with a K=1 ones matmul).
  - The output projection is computed token-major so ReduceScatter chunks
    are token-contiguous and host reassembly is a plain cast.
"""

import numpy as np

D_MODEL = 1024
S = 2048
N_CORES = 8
HPC = 4           # heads per core
COF = HPC * 64    # 256 out-features per core
TPS = S // 4      # 512 tokens per shard
WBLK = D_MODEL * COF  # 262144 elems per weight slice
WHALF = 2 * WBLK      # per-core weight half

# single merged fp16 input: [ xqkv | weight-half | identity | biases ]
OFF_X = 0
NX = 3 * TPS * D_MODEL
OFF_W = OFF_X + NX
OFF_I = OFF_W + WHALF
OFF_B = OFF_I + 128 * 128
NMEGA = OFF_B + 832

_CACHED = None


def _build():
    from concourse import bacc
    import concourse.bass as bass
    import concourse.tile as tile
    from concourse import mybir

    F16 = mybir.dt.float16
    F32R = mybir.dt.float32r
    F32 = mybir.dt.float32
    EXP = mybir.ActivationFunctionType.Exp

    nc = bacc.Bacc("TRN2", target_bir_lowering=False, debug=False,
                   num_devices=N_CORES)

    mega = nc.dram_tensor("mega", [NMEGA], F16, kind="ExternalInput")
    outp = nc.dram_tensor("outp", [TPS, D_MODEL], F16, kind="ExternalOutput")

    GROUPS4 = [[0, 1, 2, 3], [4, 5, 6, 7]]
    PAIRS = [[0, 4], [1, 5], [2, 6], [3, 7]]

    with nc.allow_low_precision(reason="fp16 transfers / f32r matmuls intended"), \
            tile.TileContext(nc) as tc:
        with (
            tc.tile_pool(name="dram", bufs=1, space="DRAM") as dram,
            tc.tile_pool(name="wconst", bufs=1) as wconst,
            tc.tile_pool(name="big", bufs=1) as big,
            tc.tile_pool(name="raw", bufs=2) as raw_pool,
            tc.tile_pool(name="qin", bufs=2) as qin_pool,
            tc.tile_pool(name="expp", bufs=4) as expp,
            tc.tile_pool(name="stage", bufs=3) as stage_pool,
            tc.tile_pool(name="bcp", bufs=2) as bcp,
            tc.tile_pool(name="small", bufs=4) as small,
            tc.tile_pool(name="psA", bufs=4, space="PSUM") as psA,
            tc.tile_pool(name="psS", bufs=2, space="PSUM") as psS,
        ):
            # ---- collective bounce buffers (DRAM) ----
            xin_b = dram.tile([3, TPS, D_MODEL], F16)    # my shard of q,k,v
            gX = dram.tile([4, 3, TPS, D_MODEL], F16)    # gathered full seq
            wb = dram.tile([WHALF], F16)                 # my weight half
            gW = dram.tile([2, WHALF], F16)              # full weight block
            ob_in = dram.tile([S, D_MODEL], F16)         # my partial out
            ob_out = dram.tile([TPS, D_MODEL], F16)      # reduced shard

            nc.sync.dma_start(
                xin_b[:],
                mega[OFF_X:OFF_X + NX].rearrange("(x t d) -> x t d",
                                                 t=TPS, d=D_MODEL))
            nc.sync.dma_start(wb[:], mega[OFF_W:OFF_W + WHALF])
            nc.gpsimd.collective_compute(
                "AllGather", mybir.AluOpType.bypass,
                replica_groups=GROUPS4,
                ins=[xin_b[:].opt()], outs=[gX[:].opt()],
            )
            nc.gpsimd.collective_compute(
                "AllGather", mybir.AluOpType.bypass,
                replica_groups=PAIRS,
                ins=[wb[:].opt()], outs=[gW[:].opt()],
            )

            # ---- weights / biases to SBUF ----
            wq_sb = wconst.tile([128, 8, COF], F16)
            wk_sb = wconst.tile([128, 8, COF], F16)
            wv_sb = wconst.tile([128, 8, COF], F16)
            wo_sb = wconst.tile([128, 2, D_MODEL], F16)
            nc.sync.dma_start(
                wq_sb[:], gW[0, 0:WBLK].rearrange("(a p f) -> p a f", p=128, f=COF))
            nc.sync.dma_start(
                wk_sb[:], gW[0, WBLK:WHALF].rearrange("(a p f) -> p a f", p=128, f=COF))
            nc.sync.dma_start(
                wv_sb[:], gW[1, 0:WBLK].rearrange("(a p f) -> p a f", p=128, f=COF))
            nc.sync.dma_start(
                wo_sb[:], gW[1, WBLK:WHALF].rearrange("(c p f) -> p c f", p=128, f=D_MODEL))

            # biases arrive fp16 inside mega; convert to f32 on device
            bqk16 = wconst.tile([128, 4], F16)
            nc.sync.dma_start(
                bqk16[:],
                mega[OFF_B:OFF_B + 512].rearrange("(p m) -> p m", m=4))
            bqk_sb = wconst.tile([128, 4], F32)  # cols: bq m0, bq m1, bk m0, bk m1
            nc.vector.tensor_copy(bqk_sb[:], bqk16[:])
            m_ap = mega[:]
            bv16 = wconst.tile([128, HPC, 64], F16)
            nc.gpsimd.dma_start(
                bv16[:],
                bass.AP(tensor=m_ap.tensor, offset=m_ap.offset + OFF_B + 512,
                        ap=[[0, 128], [64, HPC], [1, 64]]),
            )
            bv_bc = wconst.tile([128, HPC, 64], F32)
            nc.vector.tensor_copy(bv_bc[:], bv16[:])
            ones16 = wconst.tile([1, 64], F16)
            nc.sync.dma_start(
                ones16[:],
                mega[OFF_B + 768:OFF_B + 832].rearrange("(o c) -> o c", o=1))
            ones_sb = wconst.tile([1, 64], F32R)
            nc.vector.tensor_copy(ones_sb[:], ones16[:])
            onesv16 = wconst.tile([128, HPC, 1], F16)
            nc.gpsimd.dma_start(
                onesv16[:],
                bass.AP(tensor=m_ap.tensor, offset=m_ap.offset + OFF_B + 768,
                        ap=[[0, 128], [0, HPC], [1, 1]]),
            )
            id_sb = wconst.tile([128, 128], F16)
            nc.sync.dma_start(
                id_sb[:],
                mega[OFF_I:OFF_I + 128 * 128].rearrange("(p f) -> p f", f=128))

            # ---- persistent activations ----
            QT_sb = big.tile([128, 2, S], F32R)   # [p, m, t]: Q^T[m*128+p, t]
            KT_sb = big.tile([128, 2, S], F32R)
            V_sb = big.tile([128, 16, HPC, 65], F32R)  # [tok%128, tok//128, h, c]
            OT_sb = big.tile([128, 2, S], F16)    # normalized attention out^T

            # V ones-column (l accumulator rides along the PV matmul)
            for tt in range(16):
                nc.vector.tensor_copy(V_sb[:, tt, :, 64:65], onesv16[:])

            # ---- per-chunk: PE-transpose raw tokens, then project ----
            def proj_chunk(qc):
                # raw [tok%128, tokblk, featblk, feat] per tensor
                xin = qin_pool.tile([128, 3, 8, TPS], F16, tag="xin",
                                    name=f"xin_{qc}")
                for t in range(3):
                    rw = raw_pool.tile([128, 4, 8, 128], F16, tag="raw",
                                       name=f"raw_{qc}_{t}")
                    nc.sync.dma_start(
                        rw[:],
                        gX[qc, t].rearrange("(tb p) (fb f) -> p tb fb f",
                                            p=128, f=128),
                    )
                    for fb in range(8):
                        ps = psA.tile([128, 512], F32, tag="ps",
                                      name=f"tp_{qc}_{t}_{fb}")
                        for tb in range(4):
                            nc.tensor.matmul(
                                ps[:, tb * 128:(tb + 1) * 128],
                                rw[:, tb, fb, :], id_sb[:],
                                start=True, stop=True,
                            )
                        nc.vector.tensor_copy(xin[:, t, fb, :], ps[:])
                # Q/K projections (feature-major psum)
                for (ti, dst) in ((0, QT_sb), (1, KT_sb)):
                    w_sb = wq_sb if ti == 0 else wk_sb
                    for m in range(2):
                        pq = psS.tile([128, 1024], F32, tag="sc",
                                      name=f"qkps_{qc}_{ti}_{m}")
                        for kt in range(8):
                            nc.tensor.matmul(
                                pq[:, 0:TPS],
                                w_sb[:, kt, m * 128:(m + 1) * 128],
                                xin[:, ti, kt, :],
                                start=(kt == 0), stop=(kt == 7),
                            )
                        nc.vector.tensor_scalar_add(
                            dst[:, m, qc * TPS:(qc + 1) * TPS], pq[:, 0:TPS],
                            bqk_sb[:, ti * 2 + m:ti * 2 + m + 1],
                        )
                # V projection (token-major psum)
                for tsub in range(4):
                    tt = qc * 4 + tsub
                    pv = psS.tile([128, 1024], F32, tag="sc",
                                  name=f"vps_{qc}_{tsub}")
                    for kt in range(8):
                        nc.tensor.matmul(
                            pv[:, 0:COF],
                            xin[:, 2, kt, tsub * 128:(tsub + 1) * 128],
                            wv_sb[:, kt, :],
                            start=(kt == 0), stop=(kt == 7),
                        )
                    nc.vector.tensor_add(
                        V_sb[:, tt, :, 0:64],
                        pv[:, 0:COF].rearrange("p (h c) -> p h c", h=HPC),
                        bv_bc[:],
                    )

            # ---- attention (baseline structure, f32r internals) ----
            def att_pass_alloc(hp, qh):
                return [[psA.tile([128, 512], F32, tag="ps",
                                  name=f"po_{hp}_{qh}_{h2}_{qcl}")
                         for qcl in range(2)] for h2 in range(2)]

            def att_ktgroup(hp, qh, po, kts):
                for kt in kts:
                    for h2 in range(2):
                        p0 = h2 * 64
                        sc = psS.tile([128, 1024], F32, tag="sc",
                                      name=f"sc_{hp}_{qh}_{kt}_{h2}")
                        for qcl in range(2):
                            qg = qh * 2 + qcl
                            nc.tensor.matmul(
                                sc[:, qcl * 512:(qcl + 1) * 512],
                                KT_sb[p0:p0 + 64, hp, kt * 128:(kt + 1) * 128],
                                QT_sb[p0:p0 + 64, hp, qg * 512:(qg + 1) * 512],
                                start=True, stop=True,
                                tile_position=(p0, 0),
                            )
                        ex = expp.tile([128, 1024], F32R, tag="ex",
                                       name=f"ex_{hp}_{qh}_{kt}_{h2}")
                        nc.scalar.activation(out=ex[:], in_=sc[:], func=EXP,
                                             scale=0.125)
                        for qcl in range(2):
                            nc.tensor.matmul(
                                po[h2][qcl][0:65, :],
                                V_sb[:, kt, hp * 2 + h2, :],
                                ex[:, qcl * 512:(qcl + 1) * 512],
                                start=(kt == 0), stop=(kt == 15),
                            )

            def att_norm(hp, qh, po):
                # OT = po[0:64] / l  (l rides in po row 64)
                for h2 in range(2):
                    for qcl in range(2):
                        qg = qh * 2 + qcl
                        p = po[h2][qcl]
                        linv = small.tile([1, 512], F32R, tag="linv",
                                          name=f"linv_{hp}_{qh}_{h2}_{qcl}")
                        nc.vector.reciprocal(linv[:], p[64:65, :])
                        bc_ps = psS.tile([64, 512], F32, tag="sc",
                                         name=f"bc_{hp}_{qh}_{h2}_{qcl}")
                        nc.tensor.matmul(
                            bc_ps[:], ones_sb[:], linv[:],
                            start=True, stop=True,
                        )
                        bc_sb = bcp.tile([64, 512], F32, tag="bc",
                                         name=f"bcs_{hp}_{qh}_{h2}_{qcl}")
                        nc.vector.tensor_copy(bc_sb[:], bc_ps[:])
                        nc.vector.tensor_mul(
                            OT_sb[h2 * 64:(h2 + 1) * 64, hp,
                                  qg * 512:(qg + 1) * 512],
                            p[0:64, :], bc_sb[:],
                        )

            def outproj_half(qh):
                # token-major partial: out[t, of] = OT[:, t].T @ wo  (256 feats)
                for tb in range(8):
                    tok0 = qh * 1024 + tb * 128
                    pg = [psA.tile([128, 512], F32, tag="ps",
                                   name=f"pg_{qh}_{tb}_{i}") for i in range(2)]
                    for ct in range(2):
                        for i in range(2):
                            nc.tensor.matmul(
                                pg[i][:],
                                OT_sb[:, ct, tok0:tok0 + 128],
                                wo_sb[:, ct, i * 512:(i + 1) * 512],
                                start=(ct == 0), stop=(ct == 1),
                            )
                    st = stage_pool.tile([128, 1024], F16, tag="st",
                                         name=f"st_{qh}_{tb}")
                    for i in range(2):
                        nc.vector.tensor_copy(st[:, i * 512:(i + 1) * 512],
                                              pg[i][:])
                    nc.sync.dma_start(ob_in[tok0:tok0 + 128, :], st[:])

            # ---- schedule (sequential; tunnel dominates, not device) ----
            for qc in range(4):
                proj_chunk(qc)
            for qh in range(2):
                for hp in range(2):
                    po = att_pass_alloc(hp, qh)
                    att_ktgroup(hp, qh, po, range(16))
                    att_norm(hp, qh, po)
                outproj_half(qh)

            nc.gpsimd.collective_compute(
                "ReduceScatter", mybir.AluOpType.add,
                replica_groups=GROUPS4,
                ins=[ob_in[:].opt()], outs=[ob_out[:].opt()],
            )
            nc.sync.dma_start(outp[:], ob_out[:])

    nc.compile()
    return nc


def _get_runner():
    """Build the bass program, cached jitted PJRT executable, and reusable
    host staging buffers once."""
    global _CACHED
    if _CACHED is not None:
        return _CACHED

    import jax
    from jax.sharding import Mesh, PartitionSpec
    from jax.experimental.shard_map import shard_map
    from concourse import mybir
    from concourse.bass2jax import (_bass_exec_p, install_neuronx_cc_hook,
                                    partition_id_tensor)

    nc = _build()
    install_neuronx_cc_hook()

    partition_name = (nc.partition_id_tensor.name
                      if nc.partition_id_tensor else None)
    in_names, out_names, out_avals, zero_shapes = [], [], [], []
    for alloc in nc.m.functions[0].allocations:
        if not isinstance(alloc, mybir.MemoryLocationSet):
            continue
        name = alloc.memorylocations[0].name
        if alloc.kind == "ExternalInput":
            if name != partition_name:
                in_names.append(name)
        elif alloc.kind == "ExternalOutput":
            shape = tuple(alloc.tensor_shape)
            dtype = mybir.dt.np(alloc.dtype)
            out_names.append(name)
            out_avals.append(jax.core.ShapedArray(shape, dtype))
            zero_shapes.append(((N_CORES * shape[0],) + shape[1:], dtype))
    n_params = len(in_names)
    n_outs = len(out_names)
    in_names_all = in_names + out_names + (
        [partition_name] if partition_name else [])

    def _body(*args):
        operands = list(args)
        if partition_name is not None:
            operands.append(partition_id_tensor())
        outs = _bass_exec_p.bind(
            *operands, out_avals=tuple(out_avals),
            in_names=tuple(in_names_all), out_names=tuple(out_names),
            lowering_input_output_aliases=(), sim_require_finite=True,
            sim_require_nnan=True, nc=nc)
        return tuple(outs)

    devices = jax.devices()[:N_CORES]
    mesh = Mesh(np.asarray(devices), ("core",))
    in_specs = (PartitionSpec("core"),) * (n_params + n_outs)
    out_specs = (PartitionSpec("core"),) * n_outs
    donate = tuple(range(n_params, n_params + n_outs))
    sharded = jax.jit(shard_map(_body, mesh=mesh, in_specs=in_specs,
                                out_specs=out_specs, check_rep=False),
                      donate_argnums=donate, keep_unused=True)

    # preallocated host staging buffer (reused across calls); the identity
    # block is constant and filled once
    mega8 = np.zeros((2, 4, NMEGA), np.float16)
    mega8[:, :, OFF_I:OFF_I + 128 * 128].reshape(2, 4, 128, 128)[:] = (
        np.eye(128, dtype=np.float16))

    _CACHED = dict(sharded=sharded, in_names=in_names,
                   zero_shapes=zero_shapes, out_names=out_names,
                   mega8=mega8, prev_out=None)
    return _CACHED


def kernel(q, k, v, w_q, b_q, w_k, b_k, w_v, b_v, w_o, b_o):
    q, k, v = (np.asarray(x, np.float32) for x in (q, k, v))
    w_q, b_q, w_k, b_k, w_v, b_v, w_o, b_o = (
        np.asarray(x, np.float32)
        for x in (w_q, b_q, w_k, b_k, w_v, b_v, w_o, b_o)
    )

    r = _get_runner()
    mega8 = r["mega8"]

    # xqkv block: [core=(b,ts)][3, 512, 1024] token-major fp16
    xview = mega8[:, :, OFF_X:OFF_X + NX].reshape(2, 4, 3, TPS, D_MODEL)
    xview[:, :, 0] = q.reshape(2, 4, TPS, D_MODEL)
    xview[:, :, 1] = k.reshape(2, 4, TPS, D_MODEL)
    xview[:, :, 2] = v.reshape(2, 4, TPS, D_MODEL)

    # weight block: full block per head group = [wq_sl|wk_sl|wv_sl|wo_sl];
    # core c gets half c//4 of its head group's block
    # (pair half 0 = wq|wk, half 1 = wv|wo)
    wf = mega8[:, :, OFF_W:OFF_W + WHALF].reshape(2, 4, 2, WBLK)
    wf[0, :, 0].reshape(4, D_MODEL, COF)[:] = (
        w_q.reshape(D_MODEL, 4, COF).transpose(1, 0, 2))
    wf[0, :, 1].reshape(4, D_MODEL, COF)[:] = (
        w_k.reshape(D_MODEL, 4, COF).transpose(1, 0, 2))
    wf[1, :, 0].reshape(4, D_MODEL, COF)[:] = (
        w_v.reshape(D_MODEL, 4, COF).transpose(1, 0, 2))
    wf[1, :, 1].reshape(4, COF, D_MODEL)[:] = w_o.reshape(4, COF, D_MODEL)

    # bias block: [0:512] bq|bk interleaved p-major [128,4], [512:768] bv
    bview = mega8[:, :, OFF_B:OFF_B + 832]
    bqk = bview[:, :, 0:512].reshape(2, 4, 128, 4)
    bqk[:, :, :, 0:2] = b_q.reshape(4, 2, 128).transpose(0, 2, 1)
    bqk[:, :, :, 2:4] = b_k.reshape(4, 2, 128).transpose(0, 2, 1)
    bview[:, :, 512:768] = b_v.reshape(4, 256)
    bview[:, :, 768:832] = 1.0

    concat_in = [mega8.reshape(N_CORES * NMEGA)]
    if r["prev_out"] is None:
        donated = [np.zeros(shape, dt) for shape, dt in r["zero_shapes"]]
    else:
        donated = [r["prev_out"]]
    out_arrs = r["sharded"](*concat_in, *donated)
    r["prev_out"] = out_arrs[0]

    # fetch shards in parallel; shard c = tokens (c%4)*512.. of batch c//4
    out = np.empty((2, S, D_MODEL), np.float32)
    out4 = out.reshape(N_CORES, TPS, D_MODEL)
    shards = sorted(out_arrs[0].addressable_shards,
                    key=lambda sh: sh.index[0].start or 0)
    for sh in shards:
        sh.data.copy_to_host_async()
    for i, sh in enumerate(shards):
        out4[i] = np.asarray(sh.data)
    out += b_o
    return out
